# revision 2
# baseline (speedup 1.0000x reference)
"""BackwardConv2D bound-propagation kernel for Trainium2 (8 NeuronCores).

Math: for each (b, spec j), propagate linear bounds backward through a
3x3 SAME conv:  out[y,x,ci] = sum_{dy,dx,co} zpad[y+dy, x+dx, co] * K'[dy,dx,co,ci]
with K'[dy,dx,co,ci] = kernel[2-dy, 2-dx, ci, co]  (TF conv2d_transpose).

Device strategy:
 - Host repacks both w_out tensors into ZT[(yy*66+xx)*64+co, uv*160+b*10+j]
   (zero-padded 66x66 ring) so N=320 rides the contiguous axis -> full-rate DMA.
 - The conv is a banded matrix A applied to ZT rows. A 128-row output block
   (= 4 out pixels x 32 ci) needs 9 contraction chunks of 128 consecutive
   input rows (= 2 in pixels x 64 co), each with a data-independent 128x128
   stationary matrix S[dy][c] built from the kernel on the host.
 - Spatial shard: core c computes out image rows y in [8c, 8c+8) for all
   (b, uv, j). Per core: 128 output blocks x 9 matmuls [K=128,M=128,N=320].
 - Bias contribution is a tiny reduction; computed on host in float64.
"""

import numpy as np

B, N_OUT, H, W, C_OUT, C_IN, KH, KW = 16, 10, 64, 64, 64, 32, 3, 3
HP, WP = H + 2, W + 2          # padded spatial dims
N_FLAT_OUT = H * W * C_OUT     # 262144
N_IN = H * W * C_IN            # 131072
NCORES = 8
NCOL = 2 * B * N_OUT           # 320   (uv, b, j)
ROWS_PER_Y = WP * C_OUT        # 4224 padded input rows per padded image row
CHUNKS_PER_Y = ROWS_PER_Y // 128   # 33
Y_PER_CORE = H // NCORES       # 8
SLABS_PER_CORE = Y_PER_CORE + 2    # 10 padded rows per core
OUT_ROWS_PER_CORE = Y_PER_CORE * W * C_IN  # 16384

# matmul dtype for data/stationary ("float32r" = full-rate fp32 path;
# flip to "float32" if numerics are off)
MM_DT = "float32r"

_LAST_RESULT = {}


def _build_stationaries(kernel: np.ndarray) -> np.ndarray:
    """9 stationary matrices S[dy*3+c][k=(dpix*64+co), m=(po*32+ci)].

    Chunk c starts at padded in-pixel x0+2c of padded row y+dy; the block's
    out pixels are x0..x0+3 of out row y.  Tap dx = (2c+dpix) - po must be
    in [0,3) to contribute;  value = K'[dy,dx,co,ci] = kernel[2-dy,2-dx,ci,co].
    """
    S = np.zeros((9, 128, 128), dtype=np.float32)
    for dy in range(3):
        for c in range(3):
            for dpix in range(2):
                for po in range(4):
                    dx = 2 * c + dpix - po
                    if 0 <= dx < 3:
                        # [co, ci] block
                        S[dy * 3 + c,
                          dpix * 64:(dpix + 1) * 64,
                          po * 32:(po + 1) * 32] = kernel[2 - dy, 2 - dx].T
    return S


def _host_pack(w_out_u: np.ndarray, w_out_l: np.ndarray) -> np.ndarray:
    """-> ZT [HP*WP*C_OUT, 320] fp32, zero padded ring, col = uv*160+b*10+j."""
    zt = np.zeros((HP * WP * C_OUT, NCOL), dtype=np.float32)
    for uv, w in enumerate((w_out_u, w_out_l)):
        # w: [B,1,N_FLAT_OUT,N_OUT] -> [B, H, W, C_OUT, N_OUT]
        wr = np.ascontiguousarray(w.reshape(B, H, W, C_OUT, N_OUT))
        # pad spatial with 1 ring -> [B, HP, WP, C_OUT, N_OUT]
        wp = np.zeros((B, HP, WP, C_OUT, N_OUT), dtype=np.float32)
        wp[:, 1:-1, 1:-1] = wr
        # -> [HP*WP*C_OUT, B, N_OUT]
        m = wp.transpose(1, 2, 3, 0, 4).reshape(HP * WP * C_OUT, B * N_OUT)
        zt[:, uv * B * N_OUT:(uv + 1) * B * N_OUT] = m
    return zt


def _build_nc():
    import concourse.bass as bass
    import concourse.mybir as mybir
    import concourse.tile as tile
    from concourse import bacc

    mmdt = getattr(mybir.dt, MM_DT)
    f32 = mybir.dt.float32

    nc = bacc.Bacc("TRN2", target_bir_lowering=False, debug=False,
                   enable_asserts=False, num_devices=NCORES)
    zt_d = nc.dram_tensor("zt", [SLABS_PER_CORE * ROWS_PER_Y, NCOL], mmdt,
                          kind="ExternalInput")
    s_d = nc.dram_tensor("smat", [9, 128, 128], mmdt, kind="ExternalInput")
    o_d = nc.dram_tensor("o", [OUT_ROWS_PER_CORE, NCOL], f32,
                         kind="ExternalOutput")

    with tile.TileContext(nc) as tc:
        with tc.tile_pool(name="smats", bufs=1) as s_pool, \
             tc.tile_pool(name="slabs", bufs=4) as slab_pool, \
             tc.tile_pool(name="outs", bufs=4) as out_pool, \
             tc.tile_pool(name="psum", bufs=4, space="PSUM") as psum_pool:

            s_tiles = []
            for i in range(9):
                st = s_pool.tile([128, 128], mmdt, tag=f"s{i}")
                nc.sync.dma_start(st[:, :], s_d.ap()[i])
                s_tiles.append(st)

            slabs = {}

            def get_slab(r):
                if r not in slabs:
                    t = slab_pool.tile([128, CHUNKS_PER_Y, NCOL], mmdt)
                    src = zt_d.ap()[r * ROWS_PER_Y:(r + 1) * ROWS_PER_Y, :] \
                        .rearrange("(c p) n -> p c n", p=128)
                    nc.sync.dma_start(t[:, :, :], src)
                    slabs[r] = t
                return slabs[r]

            for yl in range(Y_PER_CORE):
                row_slabs = [get_slab(yl + dy) for dy in range(3)]
                for xb in range(16):           # x0 = 4*xb
                    x0 = 4 * xb
                    ps = psum_pool.tile([128, NCOL], f32)
                    idx = 0
                    for dy in range(3):
                        sl = row_slabs[dy]
                        for c in range(3):
                            cc = (x0 + 2 * c) // 2
                            nc.tensor.matmul(
                                ps[:, :], s_tiles[dy * 3 + c][:, :],
                                sl[:, cc, :],
                                start=(idx == 0), stop=(idx == 8))
                            idx += 1
                    ot = out_pool.tile([128, NCOL], f32)
                    nc.vector.tensor_copy(ot[:, :], ps[:, :])
                    bi = yl * 16 + xb
                    nc.sync.dma_start(
                        o_d.ap()[bi * 128:(bi + 1) * 128, :], ot[:, :])

    nc.compile()
    return nc


def kernel(w_out_u, b_out_u, w_out_l, b_out_l, kernel, bias):
    from concourse.bass_utils import run_bass_kernel_spmd

    w_out_u = np.asarray(w_out_u, dtype=np.float32)
    w_out_l = np.asarray(w_out_l, dtype=np.float32)
    kernel = np.asarray(kernel, dtype=np.float32)
    bias = np.asarray(bias, dtype=np.float32)
    b_out_u = np.asarray(b_out_u, dtype=np.float32)
    b_out_l = np.asarray(b_out_l, dtype=np.float32)

    S = _build_stationaries(kernel)
    ZT = _host_pack(w_out_u, w_out_l)

    in_maps = []
    for core in range(NCORES):
        r0 = (core * Y_PER_CORE) * ROWS_PER_Y          # padded row 8c
        r1 = r0 + SLABS_PER_CORE * ROWS_PER_Y
        in_maps.append({
            "zt": np.ascontiguousarray(ZT[r0:r1]),
            "smat": S,
        })

    nc = _build_nc()
    import time as _time
    try:
        res = run_bass_kernel_spmd(nc, in_maps, core_ids=list(range(NCORES)),
                                   trace=True)
    except ModuleNotFoundError:
        t0 = _time.time()
        res = run_bass_kernel_spmd(nc, in_maps, core_ids=list(range(NCORES)),
                                   trace=False)
        _LAST_RESULT["wall_ns"] = int((_time.time() - t0) * 1e9)
    _LAST_RESULT["exec_time_ns"] = res.exec_time_ns
    _LAST_RESULT["trace"] = res.instructions_and_trace
    # second run for steady-state wall timing (NEFF cached)
    t0 = _time.time()
    res = run_bass_kernel_spmd(nc, in_maps, core_ids=list(range(NCORES)),
                               trace=False)
    _LAST_RESULT["wall2_ns"] = int((_time.time() - t0) * 1e9)

    # reassemble: O rows = (yl*64+x)*32+ci, col = uv*160+b*10+j
    O = np.concatenate([res.results[c]["o"] for c in range(NCORES)], axis=0)
    Of = O.reshape(N_IN, 2, B, N_OUT)
    w_u = np.ascontiguousarray(Of[:, 0].transpose(1, 0, 2))[:, None]
    w_l = np.ascontiguousarray(Of[:, 1].transpose(1, 0, 2))[:, None]

    # bias contribution on host (float64 accumulate)
    bias_full = np.tile(bias, H * W).astype(np.float64)          # [N_FLAT_OUT]
    bu = (w_out_u.reshape(B, N_FLAT_OUT, N_OUT).astype(np.float64)
          * bias_full[None, :, None]).sum(axis=1)
    bl = (w_out_l.reshape(B, N_FLAT_OUT, N_OUT).astype(np.float64)
          * bias_full[None, :, None]).sum(axis=1)
    b_u = (bu[:, None, :].astype(np.float32) + b_out_u)
    b_l = (bl[:, None, :].astype(np.float32) + b_out_l)

    return (w_u.astype(np.float32), b_u.astype(np.float32),
            w_l.astype(np.float32), b_l.astype(np.float32))
